# revision 29
# baseline (speedup 1.0000x reference)
"""AnyLoc/NetVLAD pooling kernel for 8 Trainium2 NeuronCores.

Full inputs in, full output out. Internally:
  - data-parallel over batch: core r owns samples {2r, 2r+1}
  - comp_w sharded over its OUT dim: core r owns output columns [256r, 256r+256)
  - one AllGather of the tiny intra-normalized VLAD vectors; row norms via a
    64-byte AllGather of per-core sum-square partials + on-device reduction;
    the host concatenates the per-core column slices.

Key structure (evolved from trace analysis):
  - all large streams (x, comp_w) host-cast to bf16: the device computed in
    bf16 anyway, so HBM traffic halves with identical numerics;
  - fused [pca_w.T | G] weight precomputed on host (G = pca_w.T @ conv_w.T
    gives unnormalized logits straight from x.T); per-token 1/||h|| folded
    into the softmax weights so normalized h is never materialized; n_t
    carried as a 257th h column so one matmul yields both the VLAD numerator
    and asum;
  - both samples run in one pipelined loop of 4-tile batches: the VLAD agg
    matmuls lag one batch and each sample's finalize is interleaved so the
    softmax/VLAD vector chain hides under the next batch's PE matmuls;
  - x streams in 8 pairwise-chained chunk DMAs (2 in flight: full aggregate
    bandwidth, near-in-order arrival), comp_w in 8 more behind them; all
    DMA doorbells stay off the engines that own compute-critical queues;
  - both AllGathers write Shared-address-space DRAM (faster RDH path); the
    gathered vlads return to SBUF as 8 flat per-core-block DMAs spread over
    three engines instead of one descriptor-heavy shuffle;
  - tail GEMM is 4-way column-packed (tile_position) + a selection matmul;
    row sum-square partials AllGather as 64B, reduced with a strided
    [16 x 8] load + free-dim reduce;
  - rsqrt via bit-trick+Newton on VectorE (no ACT table switches); global L2
    of the intra-normed VLAD == 8 exactly, folded as a constant;
  - a tiny warm-up AllGather absorbs the ncfw init / first-op slow path.

Hardcoded problem shape: N=16, T=2048, D=1024, P=256, K=64, OUT=2048 (f32).
"""

import sys
import types

import numpy as np

N_CORES = 8
N, T, D, P, K, OUT = 16, 2048, 1024, 256, 64, 2048
SPC = N // N_CORES          # samples per core = 2
TT = SPC * T                # tokens per core = 4096
NT = TT // 128              # 128-token tiles per core = 32
NTS = T // 128              # tiles per sample = 16
OSL = OUT // N_CORES        # output slice per core = 256
F = K * P                   # flattened VLAD dim = 16384
FC = F // 128               # f-chunks = 128
DC = D // 128               # d-chunks = 8
HZ = P + K                  # fused h|z matmul width = 320

XTILES = [2, 2, 4, 4, 4, 4, 4, 4, 4]   # x chunk sizes in 128-token tiles
XCH = len(XTILES)
CW_DB = 8                   # comp_w doorbells (16 f-chunks each)


def _install_ntff_hook():
    """Make run_bass_kernel_spmd(trace=True) usable in this container: the
    image's antenv stub lacks axon_hooks, so inject one wired to the axon .so.
    Harmless if tracing is never requested."""
    if "antenv.axon_hooks" in sys.modules:
        return
    try:
        from trn_agent_boot.trn_boot import _ntff_profile_via_ctypes

        hook = _ntff_profile_via_ctypes("/opt/axon/libaxon_pjrt.so")
    except Exception:
        hook = None
    mod = types.ModuleType("antenv.axon_hooks")
    mod.get_axon_ntff_profile_hook = lambda: hook
    mod.set_axon_ntff_profile_hook = lambda h: None
    sys.modules["antenv.axon_hooks"] = mod


_NC_CACHE = {}


def _build():
    import concourse.bacc as bacc
    import concourse.mybir as mybir
    import concourse.tile as tile
    from concourse.masks import make_identity

    f32 = mybir.dt.float32
    bf16 = mybir.dt.bfloat16
    i32 = mybir.dt.int32
    Alu = mybir.AluOpType
    Act = mybir.ActivationFunctionType

    nc = bacc.Bacc(
        "TRN2",
        target_bir_lowering=False,
        debug=False,
        enable_asserts=False,
        num_devices=N_CORES,
    )

    # ---- DRAM I/O (per-core shards; names are the in_map keys) ----
    xt_d = nc.dram_tensor("xt", [128, DC * TT], bf16, kind="ExternalInput")
    wg_d = nc.dram_tensor("wg", [128, DC * HZ], bf16, kind="ExternalInput")
    bg_d = nc.dram_tensor("bg", [1, HZ], bf16, kind="ExternalInput")
    conv_bb_d = nc.dram_tensor("conv_b_bc", [128, K], f32, kind="ExternalInput")
    cent_d = nc.dram_tensor("cent", [K, P], f32, kind="ExternalInput")
    comp_wt_d = nc.dram_tensor("comp_wt", [128, FC * OSL], bf16, kind="ExternalInput")
    comp_b_d = nc.dram_tensor("comp_b", [1, OSL], bf16, kind="ExternalInput")
    sel4_d = nc.dram_tensor("sel4", [128, N], bf16, kind="ExternalInput")
    out_d = nc.dram_tensor("out", [N, OSL], f32, kind="ExternalOutput")

    rg = [list(range(N_CORES))]

    with tile.TileContext(nc) as tc:
        with (
            tc.tile_pool(name="consts", bufs=1) as consts,
            tc.tile_pool(name="work", bufs=4) as work,
            tc.tile_pool(name="wpool", bufs=8) as wpool,
            tc.tile_pool(name="small", bufs=4) as small,
            tc.tile_pool(name="ph", bufs=2, space="PSUM") as ph_pool,
            tc.tile_pool(name="pagg", bufs=2, space="PSUM") as pagg_pool,
            tc.tile_pool(name="pmisc", bufs=2, space="PSUM") as pmisc_pool,
            tc.tile_pool(name="pout", bufs=2, space="PSUM") as pout_pool,
            tc.tile_pool(name="dram", bufs=1, space="DRAM") as dram,
        ):
            # ---- persistent SBUF tensors ----
            WG_sb = consts.tile([128, DC, HZ], bf16, tag="WG")    # [pca_w.T | G]
            cbb_sb = consts.tile([128, K], f32, tag="cbb")        # conv_b bcast
            cent_sb = consts.tile([K, P], f32, tag="cent")
            bg_sb = consts.tile([1, HZ], bf16, tag="bg")          # [pca_b | g0]
            compb_sb = consts.tile([1, OSL], bf16, tag="compb")
            ones_sb = consts.tile([1, 128], bf16, tag="ones")
            ident_sb = consts.tile([128, 128], f32, tag="ident")
            xt_sb = consts.tile([128, DC, TT], bf16, tag="xt")
            cwT_sb = consts.tile([128, FC, OSL], bf16, tag="cwT")  # comp_w.T
            h_all = consts.tile([128, NT, P + 1], bf16, tag="hall")
            z_all = consts.tile([128, NT, K], f32, tag="zall")
            u_all = consts.tile([128, NT, K], bf16, tag="uall")
            nsq_all = consts.tile([128, NT], f32, tag="nsq")
            inv_all = consts.tile([128, NT], f32, tag="inv")
            S_all = consts.tile([128, NT], f32, tag="Sall")
            mg_sb = consts.tile([128, NT], i32, tag="mg")
            it_sb = consts.tile([128, NT + 4], i32, tag="itsb")
            rt_sb = consts.tile([128, NT + 4], f32, tag="rtsb")
            rS_all = consts.tile([128, NT], f32, tag="rSall")
            vT_own = consts.tile([128, SPC, 128], bf16, tag="vTown")
            vT_all = consts.tile([128, N, 128], bf16, tag="vTall")
            sq_scr = consts.tile([128, P], bf16, tag="sqscr")
            sel_sb = consts.tile([128, N], bf16, tag="sel")
            prand_sb = consts.tile([N, N_CORES], f32, tag="prand")
            sq64_scr = consts.tile([K, P], bf16, tag="sq64")
            rno_sb = consts.tile([N, 1], f32, tag="rno")

            # DRAM bounce buffers for collectives (f32-typed views of bf16
            # bits: halves the CCE element count -> faster AllGather)
            agv_in = dram.tile([128, 128], f32, tag="agi0", name="agv_in0")
            agv_out_t = nc.dram_tensor(
                "agv_out_sh", [128 * N_CORES, 128], f32, kind="Internal",
                addr_space="Shared",
            )
            ar_in = dram.tile([N, 1], f32, tag="ari")
            ar_out_t = nc.dram_tensor(
                "ar_out_sh", [N_CORES * N, 1], f32, kind="Internal",
                addr_space="Shared",
            )
            dum_in = dram.tile([1, 4], f32, tag="dumi")
            dum_out = dram.tile([N_CORES, 4], f32, tag="dumo", name="dum_out0")

            from concourse.tile_rust import add_dep_helper

            # ---- x loads first: token-chunk major so PE can start early.
            # Host pre-tiles xt to [128, (q, dc, t)]: every DMA below is one
            # contiguous run per partition. Chunks chained pairwise (q <- q-2)
            # so ~2 are in flight: full aggregate bandwidth, near-in-order
            # arrival.
            xdma = []
            xoff = 0
            for q in range(XCH):
                ntok = XTILES[q] * 128
                t0q = xoff // DC
                ins = nc.gpsimd.dma_start(
                    xt_sb[:, :, t0q : t0q + ntok],
                    xt_d.ap()[:, xoff : xoff + DC * ntok].rearrange(
                        "k (c t) -> k c t", c=DC
                    ),
                )
                xoff += DC * ntok
                if q >= 2:
                    add_dep_helper(ins.ins, xdma[q - 2].ins, reason="x chunk chain")
                xdma.append(ins)


            # ---- const loads (sync queue; small) ----
            nc.sync.dma_start(WG_sb[:], wg_d.ap().rearrange("k (c z) -> k c z", c=DC))
            nc.sync.dma_start(cbb_sb[:], conv_bb_d.ap())
            nc.sync.dma_start(cent_sb[:], cent_d.ap())
            nc.sync.dma_start(bg_sb[:], bg_d.ap())
            nc.sync.dma_start(compb_sb[:], comp_b_d.ap())
            nc.sync.dma_start(sel_sb[:], sel4_d.ap())
            nc.vector.memset(ones_sb[:], 1.0)
            nc.vector.memset(mg_sb[:], 0x5F3759DF)
            make_identity(nc, ident_sb[:])

            def rsqrt_dve(out_ap, in_ap, scol, width, rows=128):
                """out = 1/sqrt(in) on VectorE only (bit trick + 2 Newton
                steps, ~5e-6 rel err) - avoids ACT table-set switching."""
                ti = it_sb[0:rows, scol : scol + width]
                tm = rt_sb[0:rows, scol : scol + width]
                mg = mg_sb[0:rows, 0:width]
                nc.vector.tensor_scalar(
                    ti, in_ap.bitcast(i32), 1, None, op0=Alu.logical_shift_right
                )
                nc.vector.scalar_tensor_tensor(
                    out_ap.bitcast(i32), ti, -1, mg, op0=Alu.mult, op1=Alu.add
                )
                for _ in range(2):
                    nc.vector.tensor_mul(tm, in_ap, out_ap)
                    nc.vector.tensor_mul(tm, tm, out_ap)
                    nc.vector.tensor_scalar(
                        tm, tm, -0.5, 1.5, op0=Alu.mult, op1=Alu.add
                    )
                    nc.vector.tensor_mul(out_ap, out_ap, tm)

            # ---- comp_w.T stream-in (host pre-tiled), single phase: drains
            # well before the vlad AllGather fires ----
            cw1 = []
            CPD = FC // CW_DB
            for g in range(CW_DB):
                ins = nc.gpsimd.dma_start(
                    cwT_sb[:, g * CPD : (g + 1) * CPD, :],
                    comp_wt_d.ap()[:, g * CPD * OSL : (g + 1) * CPD * OSL],
                )
                if g < 2:
                    add_dep_helper(ins.ins, xdma[XCH - 2].ins, reason="cw after x")
                    add_dep_helper(ins.ins, xdma[XCH - 1].ins, reason="cw after x")
                else:
                    add_dep_helper(ins.ins, cw1[g - 2].ins, reason="cw chain")
                cw1.append(ins)

            # warm PE while the first x chunk streams (HAM stays at 2.4 GHz)
            pjunk0 = pmisc_pool.tile([128, 128], f32, tag="pm", name="pjunk0")
            for j in range(24):
                nc.tensor.matmul(
                    pjunk0[:],
                    WG_sb[:, 0, 0:128],
                    WG_sb[:, 0, 0:128],
                    start=True,
                    stop=True,
                )

            SB = 4  # tiles per interleaved batch

            # ================= per-sample main loop ==========================
            # pass_a (PE h|z matmuls + copies + row sum-squares) interleaved
            # with pass_b (softmax weights + VLAD agg) at 4-tile granularity so
            # the vector/scalar/gpsimd chain hides under the next batch's
            # matmuls.
            # ================= merged two-sample pipeline ====================
            # 8 global batches of 4 tiles. agg matmuls lag one batch behind
            # pass_a so PE never waits on the vector chain; each sample's vlad
            # finalize (vector) is emitted right after its last agg flush and
            # its PE transposes one batch later, hiding the chain under the
            # next batches' matmuls.
            agg_ts = [
                pagg_pool.tile([K, P + 1], f32, tag="agg", name=f"agg{s}")
                for s in range(SPC)
            ]
            wts = []

            def agg_flush(keep):
                while len(wts) > keep:
                    g, w_t = wts.pop(0)
                    nc.tensor.matmul(
                        agg_ts[g // NTS][:],
                        w_t[:],
                        h_all[:, g, :],
                        start=(g % NTS == 0),
                        stop=(g % NTS == NTS - 1),
                    )

            def passa(b):
                for ti in range(SB):
                    g = b * SB + ti
                    t0 = g * 128
                    ph = ph_pool.tile([128, HZ], f32, tag="ph")
                    for dc in range(DC):
                        nc.tensor.matmul(
                            ph[:],
                            xt_sb[:, dc, t0 : t0 + 128],
                            WG_sb[:, dc, :],
                            start=(dc == 0),
                            stop=False,
                        )
                    nc.tensor.matmul(
                        ph[:], ones_sb[0:1, 0:128], bg_sb[:], start=False,
                        stop=True,
                    )
                    nc.scalar.copy(h_all[:, g, 0:P], ph[:, 0:P])
                    nc.scalar.copy(z_all[:, g, :], ph[:, P:HZ])
                    nc.vector.scalar_tensor_tensor(
                        sq_scr[:],
                        h_all[:, g, 0:P],
                        1.0,
                        h_all[:, g, 0:P],
                        op0=Alu.mult,
                        op1=Alu.mult,
                        accum_out=nsq_all[:, g : g + 1],
                    )

            def passb(b):
                g0 = b * SB
                sl = slice(g0, g0 + SB)
                rsqrt_dve(inv_all[:, sl], nsq_all[:, sl], g0, SB)
                # n_t column = nsq * (1/n) = n
                nc.vector.tensor_mul(
                    h_all[:, sl, P], nsq_all[:, sl], inv_all[:, sl]
                )
                for ti in range(SB):
                    g = g0 + ti
                    nc.vector.scalar_tensor_tensor(
                        z_all[:, g, :],
                        z_all[:, g, :],
                        inv_all[:, g : g + 1],
                        cbb_sb[:],
                        op0=Alu.mult,
                        op1=Alu.add,
                    )
                nc.scalar.activation(u_all[:, sl, :], z_all[:, sl, :], Act.Exp)
                nc.vector.reduce_sum(
                    S_all[:, sl], u_all[:, sl, :], axis=mybir.AxisListType.X
                )
                nc.vector.reciprocal(rS_all[:, sl], S_all[:, sl])
                for ti in range(SB):
                    g = g0 + ti
                    w_t = wpool.tile([128, K], bf16, tag="w")
                    nc.vector.tensor_scalar(
                        w_t[:],
                        u_all[:, g, :],
                        rS_all[:, g : g + 1],
                        inv_all[:, g : g + 1],
                        op0=Alu.mult,
                        op1=Alu.mult,
                    )
                    wts.append((g, w_t))

            def passb2(hb):
                HB = SB // 2
                g0 = hb * HB
                sl = slice(g0, g0 + HB)
                rsqrt_dve(inv_all[:, sl], nsq_all[:, sl], g0, HB)
                nc.vector.tensor_mul(
                    h_all[:, sl, P], nsq_all[:, sl], inv_all[:, sl]
                )
                for ti in range(HB):
                    g = g0 + ti
                    nc.vector.scalar_tensor_tensor(
                        z_all[:, g, :],
                        z_all[:, g, :],
                        inv_all[:, g : g + 1],
                        cbb_sb[:],
                        op0=Alu.mult,
                        op1=Alu.add,
                    )
                nc.scalar.activation(u_all[:, sl, :], z_all[:, sl, :], Act.Exp)
                nc.vector.reduce_sum(
                    S_all[:, sl], u_all[:, sl, :], axis=mybir.AxisListType.X
                )
                nc.vector.reciprocal(rS_all[:, sl], S_all[:, sl])
                for ti in range(HB):
                    g = g0 + ti
                    w_t = wpool.tile([128, K], bf16, tag="w")
                    nc.vector.tensor_scalar(
                        w_t[:],
                        u_all[:, g, :],
                        rS_all[:, g : g + 1],
                        inv_all[:, g : g + 1],
                        op0=Alu.mult,
                        op1=Alu.mult,
                    )
                    wts.append((g, w_t))

            def fin_vec(s):
                """per-sample VLAD finalize: subtract centroids, intra-norm."""
                agg_sb = work.tile([K, P + 1], f32, tag="aggsb")
                nc.scalar.copy(agg_sb[:], agg_ts[s][:])
                vlneg = work.tile([K, P], f32, tag="vlneg")
                nc.vector.scalar_tensor_tensor(
                    vlneg[:],
                    cent_sb[:],
                    agg_sb[:, P : P + 1],
                    agg_sb[:, 0:P],
                    op0=Alu.mult,
                    op1=Alu.subtract,
                )
                vsq = small.tile([K, 1], f32, tag="vsq")
                nc.vector.scalar_tensor_tensor(
                    sq64_scr[:],
                    vlneg[:],
                    1.0,
                    vlneg[:],
                    op0=Alu.mult,
                    op1=Alu.mult,
                    accum_out=vsq[:],
                )
                r_t = small.tile([K, 1], f32, tag="r")
                rsqrt_dve(r_t[:], vsq[:], NT + s, 1, rows=K)
                vn = work.tile([K, P], f32, tag="vn")
                nc.vector.tensor_scalar(
                    vn[:], vlneg[:], r_t[:], -0.125, op0=Alu.mult, op1=Alu.mult
                )
                return vn

            def fin_pe(s, vn):
                # vT[:, 2k+half] = vn[k, 128*half + :] via two PE transposes
                tlast = None
                for half in range(2):
                    ptr = pmisc_pool.tile([128, 128], f32, tag="pm", name="ptr")
                    tlast = nc.tensor.transpose(
                        ptr[0:128, 0:K],
                        vn[:, 128 * half : 128 * (half + 1)],
                        ident_sb[0:K, 0:K],
                    )
                    nc.vector.tensor_copy(
                        vT_own[:, s, half : 128 : 2], ptr[0:128, 0:K]
                    )
                return tlast

            vn0 = None
            for b in range(NT // SB):
                passa(b)
                if b == 6:
                    fin_pe(0, vn0)
                agg_flush(SB)
                if b == NT // SB - 1:
                    # split the last batch's vector chain in two so the PE
                    # tail (final aggs + s1 finalize) starts sooner
                    passb2(2 * b)
                    agg_flush(2)
                    passb2(2 * b + 1)
                else:
                    passb(b)
                if b == 4:
                    vn0 = fin_vec(0)
            agg_flush(0)
            vn1 = fin_vec(1)
            # brief PE warm-up covering the s1 finalize vector chain
            pjunk = ph_pool.tile([128, HZ], f32, tag="ph", name="pjunk")
            for j in range(50):
                nc.tensor.matmul(
                    pjunk[:],
                    WG_sb[:, 0, 0:128],
                    WG_sb[:, 0, :],
                    start=True,
                    stop=True,
                )
            t1last = fin_pe(1, vn1)

            # keep PE warm while waiting for the AllGather; ordered AFTER the
            # last vlad transpose so it can't delay the AG
            for j in range(45):
                ji = nc.tensor.matmul(
                    pjunk[:],
                    WG_sb[:, 0, 0:128],
                    WG_sb[:, 0, :],
                    start=True,
                    stop=True,
                )
                if j == 0:
                    add_dep_helper(
                        ji.ins, t1last.ins, reason="junk after last transpose"
                    )

            # ---- one AllGather of both VLADs across cores (as f32 bits) ----
            nc.sync.dma_start(agv_in[:], vT_own[:].bitcast(f32))
            nc.gpsimd.collective_compute(
                "AllGather",
                Alu.bypass,
                replica_groups=rg,
                ins=[agv_in.opt()],
                outs=[agv_out_t.ap()],
            )
            # distribute the gathered vlads into SBUF as 8 flat per-core-block
            # DMAs spread over two idle engine queues (one contiguous run per
            # partition each, vs. one descriptor-heavy shuffle DMA)
            engs = [nc.sync, nc.gpsimd, nc.scalar]
            for b in range(N_CORES):
                engs[b % 3].dma_start(
                    vT_all[:, b * SPC : (b + 1) * SPC, :],
                    agv_out_t.ap()[b * 128 : (b + 1) * 128, :]
                    .bitcast(bf16)
                    .rearrange("k (s c) -> k s c", s=SPC),
                )

            # ---- tail GEMM, 4-way column-packed: 4 concurrent matmuls in
            # distinct PE column groups accumulate into 4 partition strips;
            # a final selection matmul sums the strips ----
            pp4 = pout_pool.tile([128, OSL], f32, tag="po", name="pp4")
            for c in range(FC):
                j = c % 4
                nc.tensor.matmul(
                    pp4[32 * j : 32 * j + N, :],
                    vT_all[:, :, c],
                    cwT_sb[:, c, :],
                    start=(c < 4),
                    stop=(c >= FC - 4),
                    tile_position=(0, 32 * j),
                )
            ppsb = work.tile([128, OSL], bf16, tag="ppsb")
            nc.vector.tensor_copy(ppsb[:], pp4[:])
            pout = pout_pool.tile([N, OSL], f32, tag="po", name="pout")
            nc.tensor.matmul(pout[:], sel_sb[:], ppsb[:], start=True, stop=False)
            nc.tensor.matmul(
                pout[:],
                ones_sb[0:1, 0:N],
                compb_sb[:],
                start=False,
                stop=True,
            )

            # ---- final row norm: AllGather partial sum-squares (64 B per
            # core), reduce across cores with a selection matmul ----
            out_sl = work.tile([N, OSL], f32, tag="osl")
            nc.vector.tensor_copy(out_sl[:], pout[:])
            osqp = small.tile([N, 1], f32, tag="osqp")
            nc.vector.scalar_tensor_tensor(
                sq_scr[0:N, 0:OSL],
                out_sl[:],
                1.0,
                out_sl[:],
                op0=Alu.mult,
                op1=Alu.mult,
                accum_out=osqp[:],
            )
            nc.sync.dma_start(ar_in[:], osqp[:])
            nc.gpsimd.collective_compute(
                "AllGather",
                Alu.bypass,
                replica_groups=rg,
                ins=[ar_in.opt()],
                outs=[ar_out_t.ap()],
            )
            # keep PE warm during the partials AG (in PE order: after the tail
            # GEMM, before the reduction matmul)
            pjunk2 = pmisc_pool.tile([128, 128], f32, tag="pm", name="pjunk2")
            for j in range(25):
                nc.tensor.matmul(
                    pjunk2[:],
                    WG_sb[:, 0, 0:128],
                    WG_sb[:, 0, 0:128],
                    start=True,
                    stop=True,
                )
            nc.sync.dma_start(
                prand_sb[:],
                ar_out_t.ap().rearrange("(c n) o -> n (c o)", n=N),
            )
            osq_sb = small.tile([N, 1], f32, tag="osq")
            nc.vector.reduce_sum(
                osq_sb[:], prand_sb[:], axis=mybir.AxisListType.X
            )
            rsqrt_dve(rno_sb[:], osq_sb[:], NT + 2, 1, rows=N)
            of = work.tile([N, OSL], f32, tag="of")
            nc.vector.tensor_scalar(of[:], out_sl[:], rno_sb[:], None, op0=Alu.mult)
            nc.sync.dma_start(out_d.ap(), of[:])

    nc.compile()
    return nc


def _get_nc():
    if "nc" not in _NC_CACHE:
        _install_ntff_hook()
        _NC_CACHE["nc"] = _build()
    return _NC_CACHE["nc"]


def kernel(**inputs):
    import ml_dtypes

    bf16 = ml_dtypes.bfloat16

    x = np.asarray(inputs["x"], dtype=np.float32)
    pca_w = np.asarray(inputs["pca_w"], dtype=np.float32)
    pca_b = np.asarray(inputs["pca_b"], dtype=np.float32)
    conv_w = np.asarray(inputs["conv_w"], dtype=np.float32)
    conv_b = np.asarray(inputs["conv_b"], dtype=np.float32)
    cent = np.asarray(inputs["centroids"], dtype=np.float32)
    comp_w = np.asarray(inputs["comp_w"], dtype=np.float32)
    comp_b = np.asarray(inputs["comp_b"], dtype=np.float32)

    nc = _get_nc()
    from concourse.bass_utils import run_bass_kernel_spmd

    # host-side layout prep (slicing / transposition / dtype fold). All bf16
    # casts mirror casts the device kernel performed anyway.
    pca_wt = pca_w.T                                             # [D, P]
    G = pca_wt.astype(bf16).astype(np.float32) @ conv_w.T.astype(bf16).astype(
        np.float32
    )                                                            # [D, K]
    wg = np.concatenate([pca_wt, G], axis=1)                     # [D, HZ]
    wg = np.ascontiguousarray(
        wg.reshape(DC, 128, HZ).transpose(1, 0, 2).reshape(128, DC * HZ)
    ).astype(bf16)
    g0 = pca_b.astype(bf16).astype(np.float32) @ conv_w.T.astype(bf16).astype(
        np.float32
    )                                                            # [K]
    bgrow = np.concatenate([pca_b, g0]).reshape(1, HZ).astype(bf16)
    conv_b_bc = np.ascontiguousarray(
        np.broadcast_to(conv_b, (128, K))
    ).astype(np.float32)
    xt = x.transpose(0, 2, 1)                                    # [N, D, T]

    sel4 = np.zeros((128, N), dtype=np.float32)
    for p in range(128):
        if p % 32 < N:
            sel4[p, p % 32] = 1.0
    sel4 = sel4.astype(bf16)

    in_maps = []
    for r in range(N_CORES):
        xt_r = np.concatenate([xt[SPC * r + j] for j in range(SPC)], axis=1)  # [D, TT]
        # pre-tile block-major [128, (q, dc, t)] so each x-chunk DMA is one
        # contiguous run per partition (full HBM rate); chunk sizes follow
        # XTILES (smaller leading chunks so the PE pipeline ramps sooner)
        xr3 = xt_r.reshape(DC, 128, TT)
        blocks, tk = [], 0
        for nt_q in XTILES:
            ntok = nt_q * 128
            blocks.append(
                xr3[:, :, tk : tk + ntok].transpose(1, 0, 2).reshape(128, DC * ntok)
            )
            tk += ntok
        xt_r = np.ascontiguousarray(np.concatenate(blocks, axis=1)).astype(bf16)
        comp_wt_r = comp_w[r * OSL : (r + 1) * OSL].T            # [F, OSL]
        comp_wt_r = np.ascontiguousarray(
            comp_wt_r.reshape(FC, 128, OSL).transpose(1, 0, 2).reshape(128, FC * OSL)
        ).astype(bf16)
        comp_b_r = comp_b[r * OSL : (r + 1) * OSL].reshape(1, OSL).astype(bf16)
        in_maps.append(
            {
                "sel4": sel4,
                "xt": xt_r,
                "wg": wg,
                "bg": bgrow,
                "conv_b_bc": conv_b_bc,
                "cent": cent,
                "comp_wt": comp_wt_r,
                "comp_b": comp_b_r,
            }
        )

    res = run_bass_kernel_spmd(nc, in_maps, core_ids=list(range(N_CORES)))
    kernel.last_results = res
    out = np.empty((N, OUT), dtype=np.float32)
    for r in range(N_CORES):
        out[:, r * OSL : (r + 1) * OSL] = np.asarray(res.results[r]["out"])
    return out


# revision 30
# speedup vs baseline: 1.0213x; 1.0213x over previous
"""AnyLoc/NetVLAD pooling kernel for 8 Trainium2 NeuronCores.

Full inputs in, full output out. Internally:
  - data-parallel over batch: core r owns samples {2r, 2r+1}
  - comp_w sharded over its OUT dim: core r owns output columns [256r, 256r+256)
  - one AllGather of the tiny intra-normalized VLAD vectors; row norms via a
    64-byte AllGather of per-core sum-square partials + on-device reduction;
    the host concatenates the per-core column slices.

Key structure (evolved from trace analysis):
  - all large streams (x, comp_w) host-cast to bf16: the device computed in
    bf16 anyway, so HBM traffic halves with identical numerics;
  - fused [pca_w.T | G] weight precomputed on host (G = pca_w.T @ conv_w.T
    gives unnormalized logits straight from x.T); per-token 1/||h|| folded
    into the softmax weights so normalized h is never materialized; n_t
    carried as a 257th h column so one matmul yields both the VLAD numerator
    and asum;
  - both samples run in one pipelined loop of 4-tile batches: the VLAD agg
    matmuls lag one batch and each sample's finalize is interleaved so the
    softmax/VLAD vector chain hides under the next batch's PE matmuls;
  - x streams in 8 pairwise-chained chunk DMAs (2 in flight: full aggregate
    bandwidth, near-in-order arrival), comp_w in 8 more behind them; all
    DMA doorbells stay off the engines that own compute-critical queues;
  - both AllGathers write Shared-address-space DRAM (faster RDH path); the
    gathered vlads return to SBUF as 8 flat per-core-block DMAs spread over
    three engines instead of one descriptor-heavy shuffle;
  - tail GEMM is 4-way column-packed (tile_position) + a selection matmul;
    row sum-square partials AllGather as 64B, reduced with a strided
    [16 x 8] load + free-dim reduce;
  - rsqrt via bit-trick+Newton on VectorE (no ACT table switches); global L2
    of the intra-normed VLAD == 8 exactly, folded as a constant;
  - a tiny warm-up AllGather absorbs the ncfw init / first-op slow path.

Hardcoded problem shape: N=16, T=2048, D=1024, P=256, K=64, OUT=2048 (f32).
"""

import sys
import types

import numpy as np

N_CORES = 8
N, T, D, P, K, OUT = 16, 2048, 1024, 256, 64, 2048
SPC = N // N_CORES          # samples per core = 2
TT = SPC * T                # tokens per core = 4096
NT = TT // 128              # 128-token tiles per core = 32
NTS = T // 128              # tiles per sample = 16
OSL = OUT // N_CORES        # output slice per core = 256
F = K * P                   # flattened VLAD dim = 16384
FC = F // 128               # f-chunks = 128
DC = D // 128               # d-chunks = 8
HZ = P + K                  # fused h|z matmul width = 320

XTILES = [2, 2, 4, 4, 4, 4, 4, 4, 4]   # x chunk sizes in 128-token tiles
XCH = len(XTILES)
CW_DB = 8                   # comp_w doorbells (16 f-chunks each)


def _install_ntff_hook():
    """Make run_bass_kernel_spmd(trace=True) usable in this container: the
    image's antenv stub lacks axon_hooks, so inject one wired to the axon .so.
    Harmless if tracing is never requested."""
    if "antenv.axon_hooks" in sys.modules:
        return
    try:
        from trn_agent_boot.trn_boot import _ntff_profile_via_ctypes

        hook = _ntff_profile_via_ctypes("/opt/axon/libaxon_pjrt.so")
    except Exception:
        hook = None
    mod = types.ModuleType("antenv.axon_hooks")
    mod.get_axon_ntff_profile_hook = lambda: hook
    mod.set_axon_ntff_profile_hook = lambda h: None
    sys.modules["antenv.axon_hooks"] = mod


_NC_CACHE = {}


def _build():
    import concourse.bacc as bacc
    import concourse.mybir as mybir
    import concourse.tile as tile
    from concourse.masks import make_identity

    f32 = mybir.dt.float32
    bf16 = mybir.dt.bfloat16
    i32 = mybir.dt.int32
    Alu = mybir.AluOpType
    Act = mybir.ActivationFunctionType

    nc = bacc.Bacc(
        "TRN2",
        target_bir_lowering=False,
        debug=False,
        enable_asserts=False,
        num_devices=N_CORES,
    )

    # ---- DRAM I/O (per-core shards; names are the in_map keys) ----
    xt_d = nc.dram_tensor("xt", [128, DC * TT], bf16, kind="ExternalInput")
    wg_d = nc.dram_tensor("wg", [128, DC * HZ], bf16, kind="ExternalInput")
    bg_d = nc.dram_tensor("bg", [1, HZ], bf16, kind="ExternalInput")
    conv_bb_d = nc.dram_tensor("conv_b_bc", [128, K], f32, kind="ExternalInput")
    cent_d = nc.dram_tensor("cent", [K, P], f32, kind="ExternalInput")
    comp_wt_d = nc.dram_tensor("comp_wt", [128, FC * OSL], bf16, kind="ExternalInput")
    comp_b_d = nc.dram_tensor("comp_b", [1, OSL], bf16, kind="ExternalInput")
    sel4_d = nc.dram_tensor("sel4", [128, N], bf16, kind="ExternalInput")
    out_d = nc.dram_tensor("out", [N, OSL], f32, kind="ExternalOutput")

    rg = [list(range(N_CORES))]

    with tile.TileContext(nc) as tc:
        with (
            tc.tile_pool(name="consts", bufs=1) as consts,
            tc.tile_pool(name="work", bufs=4) as work,
            tc.tile_pool(name="wpool", bufs=8) as wpool,
            tc.tile_pool(name="small", bufs=4) as small,
            tc.tile_pool(name="ph", bufs=2, space="PSUM") as ph_pool,
            tc.tile_pool(name="pagg", bufs=2, space="PSUM") as pagg_pool,
            tc.tile_pool(name="pmisc", bufs=2, space="PSUM") as pmisc_pool,
            tc.tile_pool(name="pout", bufs=2, space="PSUM") as pout_pool,
            tc.tile_pool(name="dram", bufs=1, space="DRAM") as dram,
        ):
            # ---- persistent SBUF tensors ----
            WG_sb = consts.tile([128, DC, HZ], bf16, tag="WG")    # [pca_w.T | G]
            cbb_sb = consts.tile([128, K], f32, tag="cbb")        # conv_b bcast
            cent_sb = consts.tile([K, P], f32, tag="cent")
            bg_sb = consts.tile([1, HZ], bf16, tag="bg")          # [pca_b | g0]
            compb_sb = consts.tile([1, OSL], bf16, tag="compb")
            ones_sb = consts.tile([1, 128], bf16, tag="ones")
            ident_sb = consts.tile([128, 128], f32, tag="ident")
            xt_sb = consts.tile([128, DC, TT], bf16, tag="xt")
            cwT_sb = consts.tile([128, FC, OSL], bf16, tag="cwT")  # comp_w.T
            h_all = consts.tile([128, NT, P + 1], bf16, tag="hall")
            z_all = consts.tile([128, NT, K], f32, tag="zall")
            u_all = consts.tile([128, NT, K], bf16, tag="uall")
            nsq_all = consts.tile([128, NT], f32, tag="nsq")
            inv_all = consts.tile([128, NT], f32, tag="inv")
            S_all = consts.tile([128, NT], f32, tag="Sall")
            mg_sb = consts.tile([128, NT], i32, tag="mg")
            it_sb = consts.tile([128, NT + 4], i32, tag="itsb")
            rt_sb = consts.tile([128, NT + 4], f32, tag="rtsb")
            rS_all = consts.tile([128, NT], f32, tag="rSall")
            vT_own = consts.tile([128, SPC, 128], bf16, tag="vTown")
            vT_all = consts.tile([128, N, 128], bf16, tag="vTall")
            sq_scr = consts.tile([128, P], bf16, tag="sqscr")
            sel_sb = consts.tile([128, N], bf16, tag="sel")
            prand_sb = consts.tile([N, N_CORES], f32, tag="prand")
            sq64_scr = consts.tile([K, P], bf16, tag="sq64")
            rno_sb = consts.tile([N, 1], f32, tag="rno")

            # DRAM bounce buffers for collectives (f32-typed views of bf16
            # bits: halves the CCE element count -> faster AllGather)
            agv_in = dram.tile([128, 128], f32, tag="agi0", name="agv_in0")
            agv_out_t = nc.dram_tensor(
                "agv_out_sh", [128 * N_CORES, 128], f32, kind="Internal",
                addr_space="Shared",
            )
            ar_in = dram.tile([N, 1], f32, tag="ari")
            ar_out_t = nc.dram_tensor(
                "ar_out_sh", [N_CORES * N, 1], f32, kind="Internal",
                addr_space="Shared",
            )
            dum_in = dram.tile([1, 4], f32, tag="dumi")
            dum_out = dram.tile([N_CORES, 4], f32, tag="dumo", name="dum_out0")

            from concourse.tile_rust import add_dep_helper

            # ---- x loads first: token-chunk major so PE can start early.
            # Host pre-tiles xt to [128, (q, dc, t)]: every DMA below is one
            # contiguous run per partition. Chunks chained pairwise (q <- q-2)
            # so ~2 are in flight: full aggregate bandwidth, near-in-order
            # arrival.
            xdma = []
            xoff = 0
            for q in range(XCH):
                ntok = XTILES[q] * 128
                t0q = xoff // DC
                ins = nc.gpsimd.dma_start(
                    xt_sb[:, :, t0q : t0q + ntok],
                    xt_d.ap()[:, xoff : xoff + DC * ntok].rearrange(
                        "k (c t) -> k c t", c=DC
                    ),
                )
                xoff += DC * ntok
                if q >= 2:
                    add_dep_helper(ins.ins, xdma[q - 2].ins, reason="x chunk chain")
                xdma.append(ins)


            # ---- const loads (sync queue; small) ----
            nc.sync.dma_start(WG_sb[:], wg_d.ap().rearrange("k (c z) -> k c z", c=DC))
            nc.sync.dma_start(cbb_sb[:], conv_bb_d.ap())
            nc.sync.dma_start(cent_sb[:], cent_d.ap())
            nc.sync.dma_start(bg_sb[:], bg_d.ap())
            nc.sync.dma_start(compb_sb[:], comp_b_d.ap())
            nc.sync.dma_start(sel_sb[:], sel4_d.ap())
            nc.vector.memset(ones_sb[:], 1.0)
            nc.vector.memset(mg_sb[:], 0x5F3759DF)
            make_identity(nc, ident_sb[:])

            def rsqrt_dve(out_ap, in_ap, scol, width, rows=128):
                """out = 1/sqrt(in) on VectorE only (bit trick + 2 Newton
                steps, ~5e-6 rel err) - avoids ACT table-set switching."""
                ti = it_sb[0:rows, scol : scol + width]
                tm = rt_sb[0:rows, scol : scol + width]
                mg = mg_sb[0:rows, 0:width]
                nc.vector.tensor_scalar(
                    ti, in_ap.bitcast(i32), 1, None, op0=Alu.logical_shift_right
                )
                nc.vector.scalar_tensor_tensor(
                    out_ap.bitcast(i32), ti, -1, mg, op0=Alu.mult, op1=Alu.add
                )
                for _ in range(2):
                    nc.vector.tensor_mul(tm, in_ap, out_ap)
                    nc.vector.tensor_mul(tm, tm, out_ap)
                    nc.vector.tensor_scalar(
                        tm, tm, -0.5, 1.5, op0=Alu.mult, op1=Alu.add
                    )
                    nc.vector.tensor_mul(out_ap, out_ap, tm)

            # ---- comp_w.T stream-in (host pre-tiled), single phase: drains
            # well before the vlad AllGather fires ----
            cw1 = []
            CPD = FC // CW_DB
            for g in range(CW_DB):
                ins = nc.gpsimd.dma_start(
                    cwT_sb[:, g * CPD : (g + 1) * CPD, :],
                    comp_wt_d.ap()[:, g * CPD * OSL : (g + 1) * CPD * OSL],
                )
                if g < 2:
                    add_dep_helper(ins.ins, xdma[XCH - 2].ins, reason="cw after x")
                    add_dep_helper(ins.ins, xdma[XCH - 1].ins, reason="cw after x")
                else:
                    add_dep_helper(ins.ins, cw1[g - 2].ins, reason="cw chain")
                cw1.append(ins)

            # warm PE while the first x chunk streams (HAM stays at 2.4 GHz)
            pjunk0 = pmisc_pool.tile([128, 128], f32, tag="pm", name="pjunk0")
            for j in range(24):
                nc.tensor.matmul(
                    pjunk0[:],
                    WG_sb[:, 0, 0:128],
                    WG_sb[:, 0, 0:128],
                    start=True,
                    stop=True,
                )

            SB = 4  # tiles per interleaved batch

            # ================= per-sample main loop ==========================
            # pass_a (PE h|z matmuls + copies + row sum-squares) interleaved
            # with pass_b (softmax weights + VLAD agg) at 4-tile granularity so
            # the vector/scalar/gpsimd chain hides under the next batch's
            # matmuls.
            # ================= merged two-sample pipeline ====================
            # 8 global batches of 4 tiles. agg matmuls lag one batch behind
            # pass_a so PE never waits on the vector chain; each sample's vlad
            # finalize (vector) is emitted right after its last agg flush and
            # its PE transposes one batch later, hiding the chain under the
            # next batches' matmuls.
            agg_ts = [
                pagg_pool.tile([2 * K, P + 1], f32, tag="agg", name=f"agg{s}")
                for s in range(SPC)
            ]
            wts = []

            def agg_flush(keep):
                # alternate tiles between two 64-row PSUM strips in distinct
                # PE column groups so consecutive agg matmuls overlap
                while len(wts) > keep:
                    g, w_t = wts.pop(0)
                    loc = g % NTS
                    j = loc % 2
                    nc.tensor.matmul(
                        agg_ts[g // NTS][K * j : K * (j + 1), :],
                        w_t[:],
                        h_all[:, g, :],
                        start=(loc == j),
                        stop=(loc == NTS - 2 + j),
                        tile_position=(0, K * j),
                    )

            def passa(b):
                for ti in range(SB):
                    g = b * SB + ti
                    t0 = g * 128
                    ph = ph_pool.tile([128, HZ], f32, tag="ph")
                    for dc in range(DC):
                        nc.tensor.matmul(
                            ph[:],
                            xt_sb[:, dc, t0 : t0 + 128],
                            WG_sb[:, dc, :],
                            start=(dc == 0),
                            stop=False,
                        )
                    nc.tensor.matmul(
                        ph[:], ones_sb[0:1, 0:128], bg_sb[:], start=False,
                        stop=True,
                    )
                    nc.scalar.copy(h_all[:, g, 0:P], ph[:, 0:P])
                    nc.scalar.copy(z_all[:, g, :], ph[:, P:HZ])
                    nc.vector.scalar_tensor_tensor(
                        sq_scr[:],
                        h_all[:, g, 0:P],
                        1.0,
                        h_all[:, g, 0:P],
                        op0=Alu.mult,
                        op1=Alu.mult,
                        accum_out=nsq_all[:, g : g + 1],
                    )

            def passb(b):
                g0 = b * SB
                sl = slice(g0, g0 + SB)
                rsqrt_dve(inv_all[:, sl], nsq_all[:, sl], g0, SB)
                # n_t column = nsq * (1/n) = n
                nc.vector.tensor_mul(
                    h_all[:, sl, P], nsq_all[:, sl], inv_all[:, sl]
                )
                for ti in range(SB):
                    g = g0 + ti
                    nc.vector.scalar_tensor_tensor(
                        z_all[:, g, :],
                        z_all[:, g, :],
                        inv_all[:, g : g + 1],
                        cbb_sb[:],
                        op0=Alu.mult,
                        op1=Alu.add,
                    )
                nc.scalar.activation(u_all[:, sl, :], z_all[:, sl, :], Act.Exp)
                nc.vector.reduce_sum(
                    S_all[:, sl], u_all[:, sl, :], axis=mybir.AxisListType.X
                )
                nc.vector.reciprocal(rS_all[:, sl], S_all[:, sl])
                for ti in range(SB):
                    g = g0 + ti
                    w_t = wpool.tile([128, K], bf16, tag="w")
                    nc.vector.tensor_scalar(
                        w_t[:],
                        u_all[:, g, :],
                        rS_all[:, g : g + 1],
                        inv_all[:, g : g + 1],
                        op0=Alu.mult,
                        op1=Alu.mult,
                    )
                    wts.append((g, w_t))

            def passb2(hb):
                HB = SB // 2
                g0 = hb * HB
                sl = slice(g0, g0 + HB)
                rsqrt_dve(inv_all[:, sl], nsq_all[:, sl], g0, HB)
                nc.vector.tensor_mul(
                    h_all[:, sl, P], nsq_all[:, sl], inv_all[:, sl]
                )
                for ti in range(HB):
                    g = g0 + ti
                    nc.vector.scalar_tensor_tensor(
                        z_all[:, g, :],
                        z_all[:, g, :],
                        inv_all[:, g : g + 1],
                        cbb_sb[:],
                        op0=Alu.mult,
                        op1=Alu.add,
                    )
                nc.scalar.activation(u_all[:, sl, :], z_all[:, sl, :], Act.Exp)
                nc.vector.reduce_sum(
                    S_all[:, sl], u_all[:, sl, :], axis=mybir.AxisListType.X
                )
                nc.vector.reciprocal(rS_all[:, sl], S_all[:, sl])
                for ti in range(HB):
                    g = g0 + ti
                    w_t = wpool.tile([128, K], bf16, tag="w")
                    nc.vector.tensor_scalar(
                        w_t[:],
                        u_all[:, g, :],
                        rS_all[:, g : g + 1],
                        inv_all[:, g : g + 1],
                        op0=Alu.mult,
                        op1=Alu.mult,
                    )
                    wts.append((g, w_t))

            def fin_vec(s):
                """per-sample VLAD finalize: subtract centroids, intra-norm."""
                agg_sb = work.tile([K, P + 1], f32, tag="aggsb")
                nc.scalar.copy(agg_sb[:], agg_ts[s][0:K, :])
                nc.vector.tensor_tensor(
                    agg_sb[:], agg_ts[s][K : 2 * K, :], agg_sb[:], op=Alu.add
                )
                vlneg = work.tile([K, P], f32, tag="vlneg")
                nc.vector.scalar_tensor_tensor(
                    vlneg[:],
                    cent_sb[:],
                    agg_sb[:, P : P + 1],
                    agg_sb[:, 0:P],
                    op0=Alu.mult,
                    op1=Alu.subtract,
                )
                vsq = small.tile([K, 1], f32, tag="vsq")
                nc.vector.scalar_tensor_tensor(
                    sq64_scr[:],
                    vlneg[:],
                    1.0,
                    vlneg[:],
                    op0=Alu.mult,
                    op1=Alu.mult,
                    accum_out=vsq[:],
                )
                r_t = small.tile([K, 1], f32, tag="r")
                rsqrt_dve(r_t[:], vsq[:], NT + s, 1, rows=K)
                vn = work.tile([K, P], f32, tag="vn")
                nc.vector.tensor_scalar(
                    vn[:], vlneg[:], r_t[:], -0.125, op0=Alu.mult, op1=Alu.mult
                )
                return vn

            def fin_pe(s, vn):
                # vT[:, 2k+half] = vn[k, 128*half + :] via two PE transposes
                tlast = None
                for half in range(2):
                    ptr = pmisc_pool.tile([128, 128], f32, tag="pm", name="ptr")
                    tlast = nc.tensor.transpose(
                        ptr[0:128, 0:K],
                        vn[:, 128 * half : 128 * (half + 1)],
                        ident_sb[0:K, 0:K],
                    )
                    nc.vector.tensor_copy(
                        vT_own[:, s, half : 128 : 2], ptr[0:128, 0:K]
                    )
                return tlast

            vn0 = None
            for b in range(NT // SB):
                passa(b)
                if b == 6:
                    fin_pe(0, vn0)
                agg_flush(SB)
                if b == NT // SB - 1:
                    # split the last batch's vector chain in two so the PE
                    # tail (final aggs + s1 finalize) starts sooner
                    passb2(2 * b)
                    agg_flush(2)
                    passb2(2 * b + 1)
                else:
                    passb(b)
                if b == 4:
                    vn0 = fin_vec(0)
            agg_flush(0)
            vn1 = fin_vec(1)
            # brief PE warm-up covering the s1 finalize vector chain
            pjunk = ph_pool.tile([128, HZ], f32, tag="ph", name="pjunk")
            for j in range(50):
                nc.tensor.matmul(
                    pjunk[:],
                    WG_sb[:, 0, 0:128],
                    WG_sb[:, 0, :],
                    start=True,
                    stop=True,
                )
            t1last = fin_pe(1, vn1)

            # keep PE warm while waiting for the AllGather; ordered AFTER the
            # last vlad transpose so it can't delay the AG
            for j in range(45):
                ji = nc.tensor.matmul(
                    pjunk[:],
                    WG_sb[:, 0, 0:128],
                    WG_sb[:, 0, :],
                    start=True,
                    stop=True,
                )
                if j == 0:
                    add_dep_helper(
                        ji.ins, t1last.ins, reason="junk after last transpose"
                    )

            # ---- one AllGather of both VLADs across cores (as f32 bits) ----
            nc.sync.dma_start(agv_in[:], vT_own[:].bitcast(f32))
            nc.gpsimd.collective_compute(
                "AllGather",
                Alu.bypass,
                replica_groups=rg,
                ins=[agv_in.opt()],
                outs=[agv_out_t.ap()],
            )
            # distribute the gathered vlads into SBUF as 8 flat per-core-block
            # DMAs spread over two idle engine queues (one contiguous run per
            # partition each, vs. one descriptor-heavy shuffle DMA)
            engs = [nc.sync, nc.gpsimd, nc.scalar]
            for b in range(N_CORES):
                engs[b % 3].dma_start(
                    vT_all[:, b * SPC : (b + 1) * SPC, :],
                    agv_out_t.ap()[b * 128 : (b + 1) * 128, :]
                    .bitcast(bf16)
                    .rearrange("k (s c) -> k s c", s=SPC),
                )

            # ---- tail GEMM, 4-way column-packed: 4 concurrent matmuls in
            # distinct PE column groups accumulate into 4 partition strips;
            # a final selection matmul sums the strips ----
            pp4 = pout_pool.tile([128, OSL], f32, tag="po", name="pp4")
            for c in range(FC):
                j = c % 4
                nc.tensor.matmul(
                    pp4[32 * j : 32 * j + N, :],
                    vT_all[:, :, c],
                    cwT_sb[:, c, :],
                    start=(c < 4),
                    stop=(c >= FC - 4),
                    tile_position=(0, 32 * j),
                )
            ppsb = work.tile([128, OSL], bf16, tag="ppsb")
            nc.vector.tensor_copy(ppsb[:], pp4[:])
            pout = pout_pool.tile([N, OSL], f32, tag="po", name="pout")
            nc.tensor.matmul(pout[:], sel_sb[:], ppsb[:], start=True, stop=False)
            nc.tensor.matmul(
                pout[:],
                ones_sb[0:1, 0:N],
                compb_sb[:],
                start=False,
                stop=True,
            )

            # ---- final row norm: AllGather partial sum-squares (64 B per
            # core), reduce across cores with a selection matmul ----
            out_sl = work.tile([N, OSL], f32, tag="osl")
            nc.vector.tensor_copy(out_sl[:], pout[:])
            osqp = small.tile([N, 1], f32, tag="osqp")
            nc.vector.scalar_tensor_tensor(
                sq_scr[0:N, 0:OSL],
                out_sl[:],
                1.0,
                out_sl[:],
                op0=Alu.mult,
                op1=Alu.mult,
                accum_out=osqp[:],
            )
            nc.sync.dma_start(ar_in[:], osqp[:])
            nc.gpsimd.collective_compute(
                "AllGather",
                Alu.bypass,
                replica_groups=rg,
                ins=[ar_in.opt()],
                outs=[ar_out_t.ap()],
            )
            # keep PE warm during the partials AG (in PE order: after the tail
            # GEMM, before the reduction matmul)
            pjunk2 = pmisc_pool.tile([128, 128], f32, tag="pm", name="pjunk2")
            for j in range(25):
                nc.tensor.matmul(
                    pjunk2[:],
                    WG_sb[:, 0, 0:128],
                    WG_sb[:, 0, 0:128],
                    start=True,
                    stop=True,
                )
            nc.sync.dma_start(
                prand_sb[:],
                ar_out_t.ap().rearrange("(c n) o -> n (c o)", n=N),
            )
            osq_sb = small.tile([N, 1], f32, tag="osq")
            nc.vector.reduce_sum(
                osq_sb[:], prand_sb[:], axis=mybir.AxisListType.X
            )
            rsqrt_dve(rno_sb[:], osq_sb[:], NT + 2, 1, rows=N)
            of = work.tile([N, OSL], f32, tag="of")
            nc.vector.tensor_scalar(of[:], out_sl[:], rno_sb[:], None, op0=Alu.mult)
            nc.sync.dma_start(out_d.ap(), of[:])

    nc.compile()
    return nc


def _get_nc():
    if "nc" not in _NC_CACHE:
        _install_ntff_hook()
        _NC_CACHE["nc"] = _build()
    return _NC_CACHE["nc"]


def kernel(**inputs):
    import ml_dtypes

    bf16 = ml_dtypes.bfloat16

    x = np.asarray(inputs["x"], dtype=np.float32)
    pca_w = np.asarray(inputs["pca_w"], dtype=np.float32)
    pca_b = np.asarray(inputs["pca_b"], dtype=np.float32)
    conv_w = np.asarray(inputs["conv_w"], dtype=np.float32)
    conv_b = np.asarray(inputs["conv_b"], dtype=np.float32)
    cent = np.asarray(inputs["centroids"], dtype=np.float32)
    comp_w = np.asarray(inputs["comp_w"], dtype=np.float32)
    comp_b = np.asarray(inputs["comp_b"], dtype=np.float32)

    nc = _get_nc()
    from concourse.bass_utils import run_bass_kernel_spmd

    # host-side layout prep (slicing / transposition / dtype fold). All bf16
    # casts mirror casts the device kernel performed anyway.
    pca_wt = pca_w.T                                             # [D, P]
    G = pca_wt.astype(bf16).astype(np.float32) @ conv_w.T.astype(bf16).astype(
        np.float32
    )                                                            # [D, K]
    wg = np.concatenate([pca_wt, G], axis=1)                     # [D, HZ]
    wg = np.ascontiguousarray(
        wg.reshape(DC, 128, HZ).transpose(1, 0, 2).reshape(128, DC * HZ)
    ).astype(bf16)
    g0 = pca_b.astype(bf16).astype(np.float32) @ conv_w.T.astype(bf16).astype(
        np.float32
    )                                                            # [K]
    bgrow = np.concatenate([pca_b, g0]).reshape(1, HZ).astype(bf16)
    conv_b_bc = np.ascontiguousarray(
        np.broadcast_to(conv_b, (128, K))
    ).astype(np.float32)
    xt = x.transpose(0, 2, 1)                                    # [N, D, T]

    sel4 = np.zeros((128, N), dtype=np.float32)
    for p in range(128):
        if p % 32 < N:
            sel4[p, p % 32] = 1.0
    sel4 = sel4.astype(bf16)

    in_maps = []
    for r in range(N_CORES):
        xt_r = np.concatenate([xt[SPC * r + j] for j in range(SPC)], axis=1)  # [D, TT]
        # pre-tile block-major [128, (q, dc, t)] so each x-chunk DMA is one
        # contiguous run per partition (full HBM rate); chunk sizes follow
        # XTILES (smaller leading chunks so the PE pipeline ramps sooner)
        xr3 = xt_r.reshape(DC, 128, TT)
        blocks, tk = [], 0
        for nt_q in XTILES:
            ntok = nt_q * 128
            blocks.append(
                xr3[:, :, tk : tk + ntok].transpose(1, 0, 2).reshape(128, DC * ntok)
            )
            tk += ntok
        xt_r = np.ascontiguousarray(np.concatenate(blocks, axis=1)).astype(bf16)
        comp_wt_r = comp_w[r * OSL : (r + 1) * OSL].T            # [F, OSL]
        comp_wt_r = np.ascontiguousarray(
            comp_wt_r.reshape(FC, 128, OSL).transpose(1, 0, 2).reshape(128, FC * OSL)
        ).astype(bf16)
        comp_b_r = comp_b[r * OSL : (r + 1) * OSL].reshape(1, OSL).astype(bf16)
        in_maps.append(
            {
                "sel4": sel4,
                "xt": xt_r,
                "wg": wg,
                "bg": bgrow,
                "conv_b_bc": conv_b_bc,
                "cent": cent,
                "comp_wt": comp_wt_r,
                "comp_b": comp_b_r,
            }
        )

    res = run_bass_kernel_spmd(nc, in_maps, core_ids=list(range(N_CORES)))
    kernel.last_results = res
    out = np.empty((N, OUT), dtype=np.float32)
    for r in range(N_CORES):
        out[:, r * OSL : (r + 1) * OSL] = np.asarray(res.results[r]["out"])
    return out


# revision 31
# speedup vs baseline: 1.0682x; 1.0458x over previous
"""AnyLoc/NetVLAD pooling kernel for 8 Trainium2 NeuronCores.

Full inputs in, full output out. Internally:
  - data-parallel over batch: core r owns samples {2r, 2r+1}
  - comp_w sharded over its OUT dim: core r owns output columns [256r, 256r+256)
  - one AllGather of the tiny intra-normalized VLAD vectors; row norms via a
    64-byte AllGather of per-core sum-square partials + on-device reduction;
    the host concatenates the per-core column slices.

Key structure (evolved from trace analysis):
  - all large streams (x, comp_w) host-cast to bf16: the device computed in
    bf16 anyway, so HBM traffic halves with identical numerics;
  - fused [pca_w.T | G] weight precomputed on host (G = pca_w.T @ conv_w.T
    gives unnormalized logits straight from x.T); per-token 1/||h|| folded
    into the softmax weights so normalized h is never materialized; n_t
    carried as a 257th h column so one matmul yields both the VLAD numerator
    and asum;
  - both samples run in one pipelined loop of 4-tile batches: the VLAD agg
    matmuls lag one batch and each sample's finalize is interleaved so the
    softmax/VLAD vector chain hides under the next batch's PE matmuls;
  - x streams in 8 pairwise-chained chunk DMAs (2 in flight: full aggregate
    bandwidth, near-in-order arrival), comp_w in 8 more behind them; all
    DMA doorbells stay off the engines that own compute-critical queues;
  - both AllGathers write Shared-address-space DRAM (faster RDH path); the
    gathered vlads return to SBUF as 8 flat per-core-block DMAs spread over
    three engines instead of one descriptor-heavy shuffle;
  - tail GEMM is 4-way column-packed (tile_position) + a selection matmul;
    row sum-square partials AllGather as 64B, reduced with a strided
    [16 x 8] load + free-dim reduce;
  - rsqrt via bit-trick+Newton on VectorE (no ACT table switches); global L2
    of the intra-normed VLAD == 8 exactly, folded as a constant;
  - a tiny warm-up AllGather absorbs the ncfw init / first-op slow path.

Hardcoded problem shape: N=16, T=2048, D=1024, P=256, K=64, OUT=2048 (f32).
"""

import sys
import types

import numpy as np

N_CORES = 8
N, T, D, P, K, OUT = 16, 2048, 1024, 256, 64, 2048
SPC = N // N_CORES          # samples per core = 2
TT = SPC * T                # tokens per core = 4096
NT = TT // 128              # 128-token tiles per core = 32
NTS = T // 128              # tiles per sample = 16
OSL = OUT // N_CORES        # output slice per core = 256
F = K * P                   # flattened VLAD dim = 16384
FC = F // 128               # f-chunks = 128
DC = D // 128               # d-chunks = 8
HZ = P + K                  # fused h|z matmul width = 320

XTILES = [2, 2, 4, 4, 4, 4, 4, 4, 4]   # x chunk sizes in 128-token tiles
XCH = len(XTILES)
CW_DB = 8                   # comp_w doorbells (16 f-chunks each)


def _install_ntff_hook():
    """Make run_bass_kernel_spmd(trace=True) usable in this container: the
    image's antenv stub lacks axon_hooks, so inject one wired to the axon .so.
    Harmless if tracing is never requested."""
    if "antenv.axon_hooks" in sys.modules:
        return
    try:
        from trn_agent_boot.trn_boot import _ntff_profile_via_ctypes

        hook = _ntff_profile_via_ctypes("/opt/axon/libaxon_pjrt.so")
    except Exception:
        hook = None
    mod = types.ModuleType("antenv.axon_hooks")
    mod.get_axon_ntff_profile_hook = lambda: hook
    mod.set_axon_ntff_profile_hook = lambda h: None
    sys.modules["antenv.axon_hooks"] = mod


_NC_CACHE = {}


def _build():
    import concourse.bacc as bacc
    import concourse.mybir as mybir
    import concourse.tile as tile
    from concourse.masks import make_identity

    f32 = mybir.dt.float32
    bf16 = mybir.dt.bfloat16
    i32 = mybir.dt.int32
    Alu = mybir.AluOpType
    Act = mybir.ActivationFunctionType

    nc = bacc.Bacc(
        "TRN2",
        target_bir_lowering=False,
        debug=False,
        enable_asserts=False,
        num_devices=N_CORES,
    )

    # ---- DRAM I/O (per-core shards; names are the in_map keys) ----
    xt_d = nc.dram_tensor("xt", [128, DC * TT], bf16, kind="ExternalInput")
    wg_d = nc.dram_tensor("wg", [128, DC * HZ], bf16, kind="ExternalInput")
    pbbc_d = nc.dram_tensor("pb_bc", [128, P], f32, kind="ExternalInput")
    g0bc_d = nc.dram_tensor("g0_bc", [128, K], f32, kind="ExternalInput")
    conv_bb_d = nc.dram_tensor("conv_b_bc", [128, K], f32, kind="ExternalInput")
    cent_d = nc.dram_tensor("cent", [K, P], f32, kind="ExternalInput")
    comp_wt_d = nc.dram_tensor("comp_wt", [128, FC * OSL], bf16, kind="ExternalInput")
    comp_b_d = nc.dram_tensor("comp_b", [1, OSL], bf16, kind="ExternalInput")
    sel4_d = nc.dram_tensor("sel4", [128, N], bf16, kind="ExternalInput")
    out_d = nc.dram_tensor("out", [N, OSL], f32, kind="ExternalOutput")

    rg = [list(range(N_CORES))]

    with tile.TileContext(nc) as tc:
        with (
            tc.tile_pool(name="consts", bufs=1) as consts,
            tc.tile_pool(name="work", bufs=4) as work,
            tc.tile_pool(name="wpool", bufs=8) as wpool,
            tc.tile_pool(name="small", bufs=4) as small,
            tc.tile_pool(name="ph", bufs=2, space="PSUM") as ph_pool,
            tc.tile_pool(name="pagg", bufs=2, space="PSUM") as pagg_pool,
            tc.tile_pool(name="pmisc", bufs=2, space="PSUM") as pmisc_pool,
            tc.tile_pool(name="pout", bufs=2, space="PSUM") as pout_pool,
            tc.tile_pool(name="dram", bufs=1, space="DRAM") as dram,
        ):
            # ---- persistent SBUF tensors ----
            WG_sb = consts.tile([128, DC, HZ], bf16, tag="WG")    # [pca_w.T | G]
            cbb_sb = consts.tile([128, K], f32, tag="cbb")        # conv_b bcast
            cent_sb = consts.tile([K, P], f32, tag="cent")
            pbbc_sb = consts.tile([128, P], f32, tag="pbbc")      # pca_b bcast
            g0bc_sb = consts.tile([128, K], f32, tag="g0bc")      # g0 bcast
            compb_sb = consts.tile([1, OSL], bf16, tag="compb")
            ones_sb = consts.tile([1, 128], bf16, tag="ones")
            ident_sb = consts.tile([128, 128], f32, tag="ident")
            xt_sb = consts.tile([128, DC, TT], bf16, tag="xt")
            cwT_sb = consts.tile([128, FC, OSL], bf16, tag="cwT")  # comp_w.T
            h_all = consts.tile([128, NT, P + 1], bf16, tag="hall")
            z_all = consts.tile([128, NT, K], f32, tag="zall")
            u_all = consts.tile([128, NT, K], bf16, tag="uall")
            nsq_all = consts.tile([128, NT], f32, tag="nsq")
            inv_all = consts.tile([128, NT], f32, tag="inv")
            S_all = consts.tile([128, NT], f32, tag="Sall")
            mg_sb = consts.tile([128, NT], i32, tag="mg")
            it_sb = consts.tile([128, NT + 4], i32, tag="itsb")
            rt_sb = consts.tile([128, NT + 4], f32, tag="rtsb")
            rS_all = consts.tile([128, NT], f32, tag="rSall")
            vT_own = consts.tile([128, SPC, 128], bf16, tag="vTown")
            vT_all = consts.tile([128, N, 128], bf16, tag="vTall")
            sq_scr = consts.tile([128, P], bf16, tag="sqscr")
            sel_sb = consts.tile([128, N], bf16, tag="sel")
            prand_sb = consts.tile([N, N_CORES], f32, tag="prand")
            sq64_scr = consts.tile([K, P], bf16, tag="sq64")
            rno_sb = consts.tile([N, 1], f32, tag="rno")

            # DRAM bounce buffers for collectives (f32-typed views of bf16
            # bits: halves the CCE element count -> faster AllGather)
            agv_in = dram.tile([128, 128], f32, tag="agi0", name="agv_in0")
            agv_out_t = nc.dram_tensor(
                "agv_out_sh", [128 * N_CORES, 128], f32, kind="Internal",
                addr_space="Shared",
            )
            ar_in = dram.tile([N, 1], f32, tag="ari")
            ar_out_t = nc.dram_tensor(
                "ar_out_sh", [N_CORES * N, 1], f32, kind="Internal",
                addr_space="Shared",
            )
            dum_in = dram.tile([1, 4], f32, tag="dumi")
            dum_out = dram.tile([N_CORES, 4], f32, tag="dumo", name="dum_out0")

            from concourse.tile_rust import add_dep_helper

            # ---- x loads first: token-chunk major so PE can start early.
            # Host pre-tiles xt to [128, (q, dc, t)]: every DMA below is one
            # contiguous run per partition. Chunks chained pairwise (q <- q-2)
            # so ~2 are in flight: full aggregate bandwidth, near-in-order
            # arrival.
            xdma = []
            xoff = 0
            for q in range(XCH):
                ntok = XTILES[q] * 128
                t0q = xoff // DC
                ins = nc.gpsimd.dma_start(
                    xt_sb[:, :, t0q : t0q + ntok],
                    xt_d.ap()[:, xoff : xoff + DC * ntok].rearrange(
                        "k (c t) -> k c t", c=DC
                    ),
                )
                xoff += DC * ntok
                if q >= 2:
                    add_dep_helper(ins.ins, xdma[q - 2].ins, reason="x chunk chain")
                xdma.append(ins)


            # ---- const loads (sync queue; small) ----
            nc.sync.dma_start(WG_sb[:], wg_d.ap().rearrange("k (c z) -> k c z", c=DC))
            nc.sync.dma_start(cbb_sb[:], conv_bb_d.ap())
            nc.sync.dma_start(cent_sb[:], cent_d.ap())
            nc.sync.dma_start(pbbc_sb[:], pbbc_d.ap())
            nc.sync.dma_start(g0bc_sb[:], g0bc_d.ap())
            nc.sync.dma_start(compb_sb[:], comp_b_d.ap())
            nc.sync.dma_start(sel_sb[:], sel4_d.ap())
            nc.vector.memset(ones_sb[:], 1.0)
            nc.vector.memset(mg_sb[:], 0x5F3759DF)
            make_identity(nc, ident_sb[:])

            def rsqrt_dve(out_ap, in_ap, scol, width, rows=128):
                """out = 1/sqrt(in) on VectorE only (bit trick + 2 Newton
                steps, ~5e-6 rel err) - avoids ACT table-set switching."""
                ti = it_sb[0:rows, scol : scol + width]
                tm = rt_sb[0:rows, scol : scol + width]
                mg = mg_sb[0:rows, 0:width]
                nc.vector.tensor_scalar(
                    ti, in_ap.bitcast(i32), 1, None, op0=Alu.logical_shift_right
                )
                nc.vector.scalar_tensor_tensor(
                    out_ap.bitcast(i32), ti, -1, mg, op0=Alu.mult, op1=Alu.add
                )
                for _ in range(2):
                    nc.vector.tensor_mul(tm, in_ap, out_ap)
                    nc.vector.tensor_mul(tm, tm, out_ap)
                    nc.vector.tensor_scalar(
                        tm, tm, -0.5, 1.5, op0=Alu.mult, op1=Alu.add
                    )
                    nc.vector.tensor_mul(out_ap, out_ap, tm)

            # ---- comp_w.T stream-in (host pre-tiled), single phase: drains
            # well before the vlad AllGather fires ----
            cw1 = []
            CPD = FC // CW_DB
            for g in range(CW_DB):
                ins = nc.gpsimd.dma_start(
                    cwT_sb[:, g * CPD : (g + 1) * CPD, :],
                    comp_wt_d.ap()[:, g * CPD * OSL : (g + 1) * CPD * OSL],
                )
                if g < 2:
                    add_dep_helper(ins.ins, xdma[XCH - 2].ins, reason="cw after x")
                    add_dep_helper(ins.ins, xdma[XCH - 1].ins, reason="cw after x")
                else:
                    add_dep_helper(ins.ins, cw1[g - 2].ins, reason="cw chain")
                cw1.append(ins)

            # warm PE while the first x chunk streams (HAM stays at 2.4 GHz)
            pjunk0 = pmisc_pool.tile([128, 128], f32, tag="pm", name="pjunk0")
            for j in range(24):
                nc.tensor.matmul(
                    pjunk0[:],
                    WG_sb[:, 0, 0:128],
                    WG_sb[:, 0, 0:128],
                    start=True,
                    stop=True,
                )

            SB = 4  # tiles per interleaved batch

            # ================= per-sample main loop ==========================
            # pass_a (PE h|z matmuls + copies + row sum-squares) interleaved
            # with pass_b (softmax weights + VLAD agg) at 4-tile granularity so
            # the vector/scalar/gpsimd chain hides under the next batch's
            # matmuls.
            # ================= merged two-sample pipeline ====================
            # 8 global batches of 4 tiles. agg matmuls lag one batch behind
            # pass_a so PE never waits on the vector chain; each sample's vlad
            # finalize (vector) is emitted right after its last agg flush and
            # its PE transposes one batch later, hiding the chain under the
            # next batches' matmuls.
            agg_ts = [
                pagg_pool.tile([2 * K, P + 1], f32, tag="agg", name=f"agg{s}")
                for s in range(SPC)
            ]
            wts = []

            def agg_flush(keep):
                # alternate tiles between two 64-row PSUM strips in distinct
                # PE column groups so consecutive agg matmuls overlap
                while len(wts) > keep:
                    g, w_t = wts.pop(0)
                    loc = g % NTS
                    j = loc % 2
                    nc.tensor.matmul(
                        agg_ts[g // NTS][K * j : K * (j + 1), :],
                        w_t[:],
                        h_all[:, g, :],
                        start=(loc == j),
                        stop=(loc == NTS - 2 + j),
                        tile_position=(0, K * j),
                    )

            def passa(b):
                for ti in range(SB):
                    g = b * SB + ti
                    t0 = g * 128
                    ph = ph_pool.tile([128, HZ], f32, tag="ph")
                    for dc in range(DC):
                        nc.tensor.matmul(
                            ph[:],
                            xt_sb[:, dc, t0 : t0 + 128],
                            WG_sb[:, dc, :],
                            start=(dc == 0),
                            stop=(dc == DC - 1),
                        )
                    nc.vector.scalar_tensor_tensor(
                        h_all[:, g, 0:P],
                        ph[:, 0:P],
                        1.0,
                        pbbc_sb[:],
                        op0=Alu.mult,
                        op1=Alu.add,
                    )
                    nc.vector.scalar_tensor_tensor(
                        z_all[:, g, :],
                        ph[:, P:HZ],
                        1.0,
                        g0bc_sb[:],
                        op0=Alu.mult,
                        op1=Alu.add,
                    )
                    nc.vector.scalar_tensor_tensor(
                        sq_scr[:],
                        h_all[:, g, 0:P],
                        1.0,
                        h_all[:, g, 0:P],
                        op0=Alu.mult,
                        op1=Alu.mult,
                        accum_out=nsq_all[:, g : g + 1],
                    )

            def passb(b):
                g0 = b * SB
                sl = slice(g0, g0 + SB)
                rsqrt_dve(inv_all[:, sl], nsq_all[:, sl], g0, SB)
                # n_t column = nsq * (1/n) = n
                nc.vector.tensor_mul(
                    h_all[:, sl, P], nsq_all[:, sl], inv_all[:, sl]
                )
                for ti in range(SB):
                    g = g0 + ti
                    nc.vector.scalar_tensor_tensor(
                        z_all[:, g, :],
                        z_all[:, g, :],
                        inv_all[:, g : g + 1],
                        cbb_sb[:],
                        op0=Alu.mult,
                        op1=Alu.add,
                    )
                nc.scalar.activation(u_all[:, sl, :], z_all[:, sl, :], Act.Exp)
                nc.vector.reduce_sum(
                    S_all[:, sl], u_all[:, sl, :], axis=mybir.AxisListType.X
                )
                nc.vector.reciprocal(rS_all[:, sl], S_all[:, sl])
                for ti in range(SB):
                    g = g0 + ti
                    w_t = wpool.tile([128, K], bf16, tag="w")
                    nc.vector.tensor_scalar(
                        w_t[:],
                        u_all[:, g, :],
                        rS_all[:, g : g + 1],
                        inv_all[:, g : g + 1],
                        op0=Alu.mult,
                        op1=Alu.mult,
                    )
                    wts.append((g, w_t))

            def passb2(hb):
                HB = SB // 2
                g0 = hb * HB
                sl = slice(g0, g0 + HB)
                rsqrt_dve(inv_all[:, sl], nsq_all[:, sl], g0, HB)
                nc.vector.tensor_mul(
                    h_all[:, sl, P], nsq_all[:, sl], inv_all[:, sl]
                )
                for ti in range(HB):
                    g = g0 + ti
                    nc.vector.scalar_tensor_tensor(
                        z_all[:, g, :],
                        z_all[:, g, :],
                        inv_all[:, g : g + 1],
                        cbb_sb[:],
                        op0=Alu.mult,
                        op1=Alu.add,
                    )
                nc.scalar.activation(u_all[:, sl, :], z_all[:, sl, :], Act.Exp)
                nc.vector.reduce_sum(
                    S_all[:, sl], u_all[:, sl, :], axis=mybir.AxisListType.X
                )
                nc.vector.reciprocal(rS_all[:, sl], S_all[:, sl])
                for ti in range(HB):
                    g = g0 + ti
                    w_t = wpool.tile([128, K], bf16, tag="w")
                    nc.vector.tensor_scalar(
                        w_t[:],
                        u_all[:, g, :],
                        rS_all[:, g : g + 1],
                        inv_all[:, g : g + 1],
                        op0=Alu.mult,
                        op1=Alu.mult,
                    )
                    wts.append((g, w_t))

            def fin_vec(s):
                """per-sample VLAD finalize: subtract centroids, intra-norm."""
                agg_sb = work.tile([K, P + 1], f32, tag="aggsb")
                nc.scalar.copy(agg_sb[:], agg_ts[s][0:K, :])
                nc.vector.tensor_tensor(
                    agg_sb[:], agg_ts[s][K : 2 * K, :], agg_sb[:], op=Alu.add
                )
                vlneg = work.tile([K, P], f32, tag="vlneg")
                nc.vector.scalar_tensor_tensor(
                    vlneg[:],
                    cent_sb[:],
                    agg_sb[:, P : P + 1],
                    agg_sb[:, 0:P],
                    op0=Alu.mult,
                    op1=Alu.subtract,
                )
                vsq = small.tile([K, 1], f32, tag="vsq")
                nc.vector.scalar_tensor_tensor(
                    sq64_scr[:],
                    vlneg[:],
                    1.0,
                    vlneg[:],
                    op0=Alu.mult,
                    op1=Alu.mult,
                    accum_out=vsq[:],
                )
                r_t = small.tile([K, 1], f32, tag="r")
                rsqrt_dve(r_t[:], vsq[:], NT + s, 1, rows=K)
                vn = work.tile([K, P], f32, tag="vn")
                nc.vector.tensor_scalar(
                    vn[:], vlneg[:], r_t[:], -0.125, op0=Alu.mult, op1=Alu.mult
                )
                return vn

            def fin_pe(s, vn):
                # vT[:, 2k+half] = vn[k, 128*half + :] via two PE transposes
                tlast = None
                for half in range(2):
                    ptr = pmisc_pool.tile([128, 128], f32, tag="pm", name="ptr")
                    tlast = nc.tensor.transpose(
                        ptr[0:128, 0:K],
                        vn[:, 128 * half : 128 * (half + 1)],
                        ident_sb[0:K, 0:K],
                    )
                    nc.vector.tensor_copy(
                        vT_own[:, s, half : 128 : 2], ptr[0:128, 0:K]
                    )
                return tlast

            vn0 = None
            for b in range(NT // SB):
                passa(b)
                if b == 6:
                    fin_pe(0, vn0)
                agg_flush(SB)
                if b == NT // SB - 1:
                    # split the last batch's vector chain in two so the PE
                    # tail (final aggs + s1 finalize) starts sooner
                    passb2(2 * b)
                    agg_flush(2)
                    passb2(2 * b + 1)
                else:
                    passb(b)
                if b == 4:
                    vn0 = fin_vec(0)
            agg_flush(0)
            vn1 = fin_vec(1)
            # brief PE warm-up covering the s1 finalize vector chain
            pjunk = ph_pool.tile([128, HZ], f32, tag="ph", name="pjunk")
            for j in range(50):
                nc.tensor.matmul(
                    pjunk[:],
                    WG_sb[:, 0, 0:128],
                    WG_sb[:, 0, :],
                    start=True,
                    stop=True,
                )
            t1last = fin_pe(1, vn1)

            # keep PE warm while waiting for the AllGather; ordered AFTER the
            # last vlad transpose so it can't delay the AG
            for j in range(45):
                ji = nc.tensor.matmul(
                    pjunk[:],
                    WG_sb[:, 0, 0:128],
                    WG_sb[:, 0, :],
                    start=True,
                    stop=True,
                )
                if j == 0:
                    add_dep_helper(
                        ji.ins, t1last.ins, reason="junk after last transpose"
                    )

            # ---- one AllGather of both VLADs across cores (as f32 bits) ----
            nc.sync.dma_start(agv_in[:], vT_own[:].bitcast(f32))
            nc.gpsimd.collective_compute(
                "AllGather",
                Alu.bypass,
                replica_groups=rg,
                ins=[agv_in.opt()],
                outs=[agv_out_t.ap()],
            )
            # distribute the gathered vlads into SBUF as 8 flat per-core-block
            # DMAs spread over two idle engine queues (one contiguous run per
            # partition each, vs. one descriptor-heavy shuffle DMA)
            engs = [nc.sync, nc.gpsimd, nc.scalar]
            for b in range(N_CORES):
                engs[b % 3].dma_start(
                    vT_all[:, b * SPC : (b + 1) * SPC, :],
                    agv_out_t.ap()[b * 128 : (b + 1) * 128, :]
                    .bitcast(bf16)
                    .rearrange("k (s c) -> k s c", s=SPC),
                )

            # ---- tail GEMM, 4-way column-packed: 4 concurrent matmuls in
            # distinct PE column groups accumulate into 4 partition strips;
            # a final selection matmul sums the strips ----
            pp4 = pout_pool.tile([128, OSL], f32, tag="po", name="pp4")
            for c in range(FC):
                j = c % 4
                nc.tensor.matmul(
                    pp4[32 * j : 32 * j + N, :],
                    vT_all[:, :, c],
                    cwT_sb[:, c, :],
                    start=(c < 4),
                    stop=(c >= FC - 4),
                    tile_position=(0, 32 * j),
                )
            ppsb = work.tile([128, OSL], bf16, tag="ppsb")
            nc.vector.tensor_copy(ppsb[:], pp4[:])
            pout = pout_pool.tile([N, OSL], f32, tag="po", name="pout")
            nc.tensor.matmul(pout[:], sel_sb[:], ppsb[:], start=True, stop=False)
            nc.tensor.matmul(
                pout[:],
                ones_sb[0:1, 0:N],
                compb_sb[:],
                start=False,
                stop=True,
            )

            # ---- final row norm: AllGather partial sum-squares (64 B per
            # core), reduce across cores with a selection matmul ----
            out_sl = work.tile([N, OSL], f32, tag="osl")
            nc.vector.tensor_copy(out_sl[:], pout[:])
            osqp = small.tile([N, 1], f32, tag="osqp")
            nc.vector.scalar_tensor_tensor(
                sq_scr[0:N, 0:OSL],
                out_sl[:],
                1.0,
                out_sl[:],
                op0=Alu.mult,
                op1=Alu.mult,
                accum_out=osqp[:],
            )
            nc.sync.dma_start(ar_in[:], osqp[:])
            nc.gpsimd.collective_compute(
                "AllGather",
                Alu.bypass,
                replica_groups=rg,
                ins=[ar_in.opt()],
                outs=[ar_out_t.ap()],
            )
            # keep PE warm during the partials AG (in PE order: after the tail
            # GEMM, before the reduction matmul)
            pjunk2 = pmisc_pool.tile([128, 128], f32, tag="pm", name="pjunk2")
            for j in range(25):
                nc.tensor.matmul(
                    pjunk2[:],
                    WG_sb[:, 0, 0:128],
                    WG_sb[:, 0, 0:128],
                    start=True,
                    stop=True,
                )
            nc.sync.dma_start(
                prand_sb[:],
                ar_out_t.ap().rearrange("(c n) o -> n (c o)", n=N),
            )
            osq_sb = small.tile([N, 1], f32, tag="osq")
            nc.vector.reduce_sum(
                osq_sb[:], prand_sb[:], axis=mybir.AxisListType.X
            )
            rsqrt_dve(rno_sb[:], osq_sb[:], NT + 2, 1, rows=N)
            of = work.tile([N, OSL], f32, tag="of")
            nc.vector.tensor_scalar(of[:], out_sl[:], rno_sb[:], None, op0=Alu.mult)
            nc.sync.dma_start(out_d.ap(), of[:])

    nc.compile()
    return nc


def _get_nc():
    if "nc" not in _NC_CACHE:
        _install_ntff_hook()
        _NC_CACHE["nc"] = _build()
    return _NC_CACHE["nc"]


def kernel(**inputs):
    import ml_dtypes

    bf16 = ml_dtypes.bfloat16

    x = np.asarray(inputs["x"], dtype=np.float32)
    pca_w = np.asarray(inputs["pca_w"], dtype=np.float32)
    pca_b = np.asarray(inputs["pca_b"], dtype=np.float32)
    conv_w = np.asarray(inputs["conv_w"], dtype=np.float32)
    conv_b = np.asarray(inputs["conv_b"], dtype=np.float32)
    cent = np.asarray(inputs["centroids"], dtype=np.float32)
    comp_w = np.asarray(inputs["comp_w"], dtype=np.float32)
    comp_b = np.asarray(inputs["comp_b"], dtype=np.float32)

    nc = _get_nc()
    from concourse.bass_utils import run_bass_kernel_spmd

    # host-side layout prep (slicing / transposition / dtype fold). All bf16
    # casts mirror casts the device kernel performed anyway.
    pca_wt = pca_w.T                                             # [D, P]
    G = pca_wt.astype(bf16).astype(np.float32) @ conv_w.T.astype(bf16).astype(
        np.float32
    )                                                            # [D, K]
    wg = np.concatenate([pca_wt, G], axis=1)                     # [D, HZ]
    wg = np.ascontiguousarray(
        wg.reshape(DC, 128, HZ).transpose(1, 0, 2).reshape(128, DC * HZ)
    ).astype(bf16)
    g0 = pca_b.astype(bf16).astype(np.float32) @ conv_w.T.astype(bf16).astype(
        np.float32
    )                                                            # [K]
    pb_bc = np.ascontiguousarray(np.broadcast_to(
        pca_b.astype(bf16).astype(np.float32), (128, P)))
    g0_bc = np.ascontiguousarray(np.broadcast_to(
        g0.astype(bf16).astype(np.float32), (128, K)))
    conv_b_bc = np.ascontiguousarray(
        np.broadcast_to(conv_b, (128, K))
    ).astype(np.float32)
    xt = x.transpose(0, 2, 1)                                    # [N, D, T]

    sel4 = np.zeros((128, N), dtype=np.float32)
    for p in range(128):
        if p % 32 < N:
            sel4[p, p % 32] = 1.0
    sel4 = sel4.astype(bf16)

    in_maps = []
    for r in range(N_CORES):
        xt_r = np.concatenate([xt[SPC * r + j] for j in range(SPC)], axis=1)  # [D, TT]
        # pre-tile block-major [128, (q, dc, t)] so each x-chunk DMA is one
        # contiguous run per partition (full HBM rate); chunk sizes follow
        # XTILES (smaller leading chunks so the PE pipeline ramps sooner)
        xr3 = xt_r.reshape(DC, 128, TT)
        blocks, tk = [], 0
        for nt_q in XTILES:
            ntok = nt_q * 128
            blocks.append(
                xr3[:, :, tk : tk + ntok].transpose(1, 0, 2).reshape(128, DC * ntok)
            )
            tk += ntok
        xt_r = np.ascontiguousarray(np.concatenate(blocks, axis=1)).astype(bf16)
        comp_wt_r = comp_w[r * OSL : (r + 1) * OSL].T            # [F, OSL]
        comp_wt_r = np.ascontiguousarray(
            comp_wt_r.reshape(FC, 128, OSL).transpose(1, 0, 2).reshape(128, FC * OSL)
        ).astype(bf16)
        comp_b_r = comp_b[r * OSL : (r + 1) * OSL].reshape(1, OSL).astype(bf16)
        in_maps.append(
            {
                "sel4": sel4,
                "xt": xt_r,
                "wg": wg,
                "pb_bc": pb_bc,
                "g0_bc": g0_bc,
                "conv_b_bc": conv_b_bc,
                "cent": cent,
                "comp_wt": comp_wt_r,
                "comp_b": comp_b_r,
            }
        )

    res = run_bass_kernel_spmd(nc, in_maps, core_ids=list(range(N_CORES)))
    kernel.last_results = res
    out = np.empty((N, OUT), dtype=np.float32)
    for r in range(N_CORES):
        out[:, r * OSL : (r + 1) * OSL] = np.asarray(res.results[r]["out"])
    return out
